# revision 16
# baseline (speedup 1.0000x reference)
"""B-spline (clamped) surface evaluation on 8 Trainium2 cores.

Math: out[u, v, :] = sum_{a,b} Bu[u,a] * Bv[v,b] * P[su[u]-p+a, sv[v]-p+b, :]

Host precomputes the tiny Cox-de-Boor basis, scatters it into dense matrices
Au [Nu, 64], Av [Nv, 64], and folds the small control-point contraction
T[u, j, d] = sum_i Au[u, i] P[i, j, d] (25M MACs, fp64 on host). The device
then does the dominant contraction (768M MACs):

  S[u, v, d] = sum_j T[u, j, d] * Av[v, j]       (TensorEngine matmuls)

The rel-err gate is 2e-2, so everything device-side runs in plain bf16
(~3e-3 total): no hi/lo split, and the output is written to HBM as bf16
(24 MB total instead of 48 MB) with the host casting back to fp32.

The K=64 contraction is zero-padded to K=128 (rows 64-127 = 0) so the
matmuls use the standard full-array config; matmul cycles scale with the
streamed column count, not K, so the padding is free.  Only the real 64
rows are DMA'd; the zero halves are memset on-device by the otherwise-idle
DVE/ACT engines during the startup window.

Latency choreography (the kernel is dependency-bound, not
throughput-bound): the NEFF start barrier gates everything until ~6.5 us,
and each input DMA pays gen + transfer + ~1.5-2 us completion receipt.  So
inputs are split into 5 independent tiles, ordered by first use, across
two parallel rings (HWDGE sync: tta, avt1, avt2; SWDGE: avt3, ttb), and 6
dummy warmup matmuls on a zeroed tile keep the PE busy from the barrier
until real data lands -- both bridging the HAM clock-gate window (PE runs
1.2 GHz until ~3.4 us of sustained activity) and wasting no warm-up time.

Each (u-tile, d) group runs LDWEIGHTS once then streams 4 x N<=512 matmuls
into two 2-bank PSUM tiles; DVE and ACT alternate evacuating them with the
fp32->bf16 cast fused into wide [128, ~1000] copies (fp32 PSUM reads run at
1 elem/cycle/lane, so wide copies amortize the ~120-170 cycle fixed cost).
The group's [128, 2001] bf16 output region (512 KB) is flushed to HBM as
soon as its two copies land, round-robined over 4 SWDGE queues, so the
output DMA (~8.3 us/core at the ~358 GB/s HBM roofline) overlaps compute;
the final group flushes in two halves to shorten the completion tail.
Flushes always cover all 128 partitions: partial-partition DMAs were
measured to unbalance the SDMA engine split ~3x.

Sharding: data-parallel over u. Each core computes a [251, 2001, 3] slab,
padded to 2x128 u-rows on device; the host drops the padding and
interleaves d.
"""

import numpy as np

N_CTRL = 64
N_EVAL = 2001
N_CORES = 8
NU_SHARD = 251   # ceil(2001 / 8); 8 * 251 = 2008 (last 7 rows are zero padding)
NU_PAD = 256     # per-core u padded to 2 full 128-wide PE column tiles
V_TILE = 512
V_HALF = 1024    # psum/copy/flush half-split of the v axis

_CACHE = {}


def _clamped_knots(p, n_ctrl, dtype=np.float64):
    n_internal = n_ctrl - p - 1
    internal = np.linspace(0.0, 1.0, n_internal + 2, dtype=dtype)[1:-1]
    return np.concatenate(
        [np.zeros(p + 1, dtype), internal, np.ones(p + 1, dtype)]
    )


def _dense_basis(params, p, n_ctrl):
    """Dense basis matrix A [len(params), n_ctrl], float64, with
    A[k, span-p+a] = B[k, a] (Cox-de-Boor, NURBS book A2.2)."""
    knots = _clamped_knots(p, n_ctrl)
    u = np.asarray(params, np.float64)
    spans = np.clip(np.searchsorted(knots, u, side="right") - 1, p, n_ctrl - 1)
    Ns = [np.ones_like(u)]
    left = {}
    right = {}
    for j in range(1, p + 1):
        left[j] = u - knots[spans + 1 - j]
        right[j] = knots[spans + j] - u
        saved = np.zeros_like(u)
        new = []
        for r in range(j):
            temp = Ns[r] / (right[r + 1] + left[j - r])
            new.append(saved + right[r + 1] * temp)
            saved = left[j - r] * temp
        new.append(saved)
        Ns = new
    B = np.stack(Ns, axis=-1)  # [N, p+1]
    A = np.zeros((len(u), n_ctrl), np.float64)
    rows = np.arange(len(u))[:, None]
    cols = spans[:, None] - p + np.arange(p + 1)[None, :]
    A[rows, cols] = B
    return A


# input tiles: name -> columns.  tta = weight group (d=0, g=0); ttb = the
# other five groups; avt1/2/3 = Av.T column ranges in first-use order.
IN_COLS = {
    "tta": 128,
    "ttb": 3 * NU_PAD - 128,
    "avt1": V_TILE,
    "avt2": V_TILE,
    "avt3": N_EVAL - V_HALF,
}


def _build_device():
    if "nc" in _CACHE:
        return _CACHE["nc"]

    import concourse.mybir as mybir
    import concourse.tile as tile
    from concourse import bacc

    f32 = mybir.dt.float32
    bf16 = mybir.dt.bfloat16
    nc = bacc.Bacc(
        "TRN2", target_bir_lowering=False, debug=False, num_devices=N_CORES,
        num_swdge_queues=4,
    )
    ins = {
        name: nc.dram_tensor(name, [64, cols], bf16, kind="ExternalInput").ap()
        for name, cols in IN_COLS.items()
    }
    # out col = g*6003 + d*2001 + v for u-tile g in {0, 1}
    out_h = nc.dram_tensor(
        "out", [128, 2 * 3 * N_EVAL], bf16, kind="ExternalOutput"
    ).ap()

    # (v0, width, avt tile, col offset within it) in stream order
    VT = [
        (0, V_TILE, "avt1", 0),
        (V_TILE, V_TILE, "avt2", 0),
        (V_HALF, V_TILE, "avt3", 0),
        (V_HALF + V_TILE, N_EVAL - V_HALF - V_TILE, "avt3", V_TILE),
    ]

    with tile.TileContext(nc) as tc:
        with (
            tc.tile_pool(name="consts", bufs=1) as consts,
            tc.tile_pool(name="ps", bufs=4, space="PSUM") as psp,
            tc.tile_pool(name="obuf", bufs=1) as obuf,
        ):
            sb = {
                name: consts.tile([128, cols], bf16, tag=name, name=name)
                for name, cols in IN_COLS.items()
            }
            # real data rows 0-63 via two parallel rings, first-use first
            for name in ("tta", "avt1", "avt2"):
                nc.sync.dma_start(out=sb[name][0:64, :], in_=ins[name])
            for qi, name in ((1, "avt3"), (2, "ttb")):
                dma = nc.gpsimd.dma_start(out=sb[name][0:64, :], in_=ins[name])
                dma.ins.queue = f"qPoolDynamic{qi}"
            # zero rows 64-127 on-device (must be finite: the PE multiplies
            # them even though the zero weight rows null the products)
            warm = consts.tile([128, V_TILE], bf16, tag="warm", name="warm")
            nc.vector.memset(warm, 0.0)
            for name in ("avt2", "avt3"):
                nc.vector.memset(sb[name][64:128, :], 0.0)
            for name in ("tta", "ttb", "avt1"):
                nc.scalar.memzero(sb[name][64:128, :])

            # PE warmup: bridge barrier-to-data-landing so the HAM clock
            # gate sees sustained activity (cold PE = 1.2 GHz for ~3.4 us)
            for _ in range(5):
                wps = psp.tile([128, V_HALF], f32, tag="ps")
                nc.tensor.matmul(
                    wps[:, :V_TILE], warm[:, :128], warm,
                    start=True, stop=True,
                )

            ob = {
                g: obuf.tile([128, 3 * N_EVAL], bf16, tag=f"ob{g}",
                             name=f"ob{g}")
                for g in range(2)
            }

            prev_mm = None
            n_out = 0
            gi = 0
            for d in range(3):
                for g in range(2):
                    col = d * NU_PAD + g * 128
                    if col == 0:
                        w = sb["tta"][:, 0:128]
                    else:
                        w = sb["ttb"][:, col - 128:col]
                    ldw = nc.tensor.ldweights(w)
                    if prev_mm is not None:
                        tile.add_dep_helper(
                            ldw.ins, prev_mm.ins, sync=False,
                            reason="weight group order",
                        )
                    # first and last groups use per-v-tile copies and
                    # half-region flushes: the first so the output DMA
                    # window opens ~1 us earlier, the last to shorten the
                    # end-of-kernel copy+transfer+receipt tail
                    fine = gi in (0, 5)
                    obase = g * 3 * N_EVAL + d * N_EVAL
                    for hi, (h0, hw) in enumerate(((0, V_HALF),
                                                   (V_HALF, N_EVAL - V_HALF))):
                        ps = psp.tile([128, V_HALF], f32, tag="ps")
                        for vi, (v0, vw, av, c0) in enumerate(
                                VT[2 * hi:2 * hi + 2]):
                            mm = nc.tensor.matmul(
                                ps[:, v0 - h0:v0 - h0 + vw], w,
                                sb[av][:, c0:c0 + vw],
                                start=True, stop=True,
                            )
                            mm.ins.ldweights = False
                            tile.add_dep_helper(
                                mm.ins, ldw.ins, sync=False,
                                reason="matmul after its ldweights",
                            )
                            prev_mm = mm
                            if fine:
                                # per-v-tile copy, engines alternating
                                osl = slice(d * N_EVAL + v0,
                                            d * N_EVAL + v0 + vw)
                                if (2 * hi + vi) % 2 == 0:
                                    nc.vector.tensor_copy(
                                        ob[g][:, osl], ps[:, v0 - h0:
                                                          v0 - h0 + vw])
                                else:
                                    nc.scalar.copy(
                                        ob[g][:, osl], ps[:, v0 - h0:
                                                          v0 - h0 + vw])
                        osl = slice(d * N_EVAL + h0, d * N_EVAL + h0 + hw)
                        if not fine:
                            # one wide copy per half; engines alternate and
                            # disjoint ranges run concurrently
                            if (hi == 0) == (gi % 2 == 0):
                                nc.vector.tensor_copy(ob[g][:, osl],
                                                      ps[:, :hw])
                            else:
                                nc.scalar.copy(ob[g][:, osl], ps[:, :hw])
                        if fine:
                            dma = nc.gpsimd.dma_start(
                                out=out_h[:, obase + h0:obase + h0 + hw],
                                in_=ob[g][:, osl],
                            )
                            dma.ins.queue = f"qPoolDynamic{n_out % 4 or ''}"
                            n_out += 1
                    if not fine:
                        # flush this (u-tile, d) region once its copies land
                        osl = slice(d * N_EVAL, (d + 1) * N_EVAL)
                        dma = nc.gpsimd.dma_start(
                            out=out_h[:, obase:obase + N_EVAL],
                            in_=ob[g][:, osl],
                        )
                        dma.ins.queue = f"qPoolDynamic{n_out % 4 or ''}"
                        n_out += 1
                    gi += 1
    nc.compile()
    _CACHE["nc"] = nc
    return nc


def kernel(control_points, params_u, params_v, degree):
    import ml_dtypes
    from concourse.bass_utils import run_bass_kernel_spmd

    p = int(np.asarray(degree))
    cp = np.asarray(control_points, np.float32)
    pu = np.asarray(params_u, np.float32)
    pv = np.asarray(params_v, np.float32)
    assert cp.shape == (N_CTRL, N_CTRL, 3), cp.shape
    assert pu.shape == (N_EVAL,) and pv.shape == (N_EVAL,), (pu.shape, pv.shape)

    Au = np.zeros((N_CORES * NU_SHARD, N_CTRL), np.float64)
    Au[:N_EVAL] = _dense_basis(pu, p, N_CTRL)
    Av = _dense_basis(pv, p, N_CTRL)

    # host stage 1 (0.3% of the FLOPs): T[j, d, u] = sum_i P[i,j,d] Au[u,i]
    T = (cp.astype(np.float64).transpose(1, 2, 0).reshape(3 * N_CTRL, N_CTRL)
         @ Au.T).reshape(N_CTRL, 3, N_CORES * NU_SHARD)

    avt = Av.T.astype(np.float32).astype(ml_dtypes.bfloat16)
    avs = {
        "avt1": np.ascontiguousarray(avt[:, :V_TILE]),
        "avt2": np.ascontiguousarray(avt[:, V_TILE:V_HALF]),
        "avt3": np.ascontiguousarray(avt[:, V_HALF:]),
    }

    nc = _build_device()
    in_maps = []
    for c in range(N_CORES):
        ttc = np.zeros((N_CTRL, 3, NU_PAD), np.float32)
        ttc[:, :, :NU_SHARD] = T[:, :, c * NU_SHARD:(c + 1) * NU_SHARD]
        tt = ttc.reshape(N_CTRL, 3 * NU_PAD).astype(ml_dtypes.bfloat16)
        in_maps.append({
            "tta": np.ascontiguousarray(tt[:, :128]),
            "ttb": np.ascontiguousarray(tt[:, 128:]),
            **avs,
        })

    res = run_bass_kernel_spmd(
        nc,
        in_maps,
        core_ids=list(range(N_CORES)),
        trace=_CACHE.get("trace", False),
        **_CACHE.get("run_kwargs", {}),
    )
    _CACHE["last_result"] = res
    # out col = g*6003 + d*2001 + v; u-tile g=1 holds rows 128..250
    full = np.empty((N_CORES * NU_SHARD, 3, N_EVAL), np.float32)
    for c, r in enumerate(res.results):
        o = np.asarray(r["out"]).astype(np.float32)
        o = o.reshape(128, 2, 3, N_EVAL)  # cols are [g][d][v] row-major
        full[c * NU_SHARD:c * NU_SHARD + 128] = o[:, 0]
        full[c * NU_SHARD + 128:(c + 1) * NU_SHARD] = o[:NU_SHARD - 128, 1]
    return np.ascontiguousarray(full[:N_EVAL].transpose(0, 2, 1))
